# revision 23
# baseline (speedup 1.0000x reference)
"""GQA causal-attention prefill (B=2, T=S=2048, D=2048, N=16, K=4, H=128)
on 8 Trainium2 NeuronCores.

Sharding: one (batch, kv-head) pair per core -> 2*4 = 8 cores, zero
cross-core communication on device; the o_proj partial sums (over each
batch's 4 kv-head groups) are reduced on the host at unshard time.

Design (vs. the f32r phase-separated baseline at 345us):
  * fp16 operands for the Q/K path (x, Wq/Wk/Wv, qt/kt, RoPE tables):
    1 cyc/row on the PE like bf16 but with 8x finer mantissa so softmax
    scores stay accurate; bf16 for pb/V/otc/Wo (pb needs bf16's range:
    exp(score) reaches ~1e25). PSUM accumulation is f32 everywhere.
  * single fused, software-pipelined loop: attention chunk c is emitted
    interleaved with the projections of chunk c+1 so the Tensor engine
    always has independent work during the softmax (ACT) phases.
  * causal mask applied by a PE matmul (ident^T @ tri -> PSUM with
    start=True; the score matmul accumulates on top with start=False).
  * softmax denominator via col-tiled [128,32]-ones matmuls (2 heads of
    a pair run concurrently in col groups 0/32 of a per-pair PSUM bank;
    the bank is pre-cleared by a zero-weight matmul because the
    col-group writes race the start=True bank-wide has_written clear).
  * reciprocal once per pair on the den bank (reciprocal_approx_fast
    must read at partition offset 0 - it ignores partition offsets);
    broadcast via K=1 f32r matmul (baseline-proven pattern).
  * PSUM budget: 2 ot + 2 score + 2 den + 2 flex = 8 banks.
"""
import sys
import types

import numpy as np
import ml_dtypes

try:  # make trace=True degrade gracefully when axon_hooks is absent
    import antenv.axon_hooks  # noqa: F401
except Exception:
    try:
        import antenv
        _m = types.ModuleType("antenv.axon_hooks")
        _h = [None]
        _m.set_axon_ntff_profile_hook = lambda h: _h.__setitem__(0, h)
        _m.get_axon_ntff_profile_hook = lambda: _h[0]
        sys.modules["antenv.axon_hooks"] = _m
        antenv.axon_hooks = _m
    except Exception:
        pass

import concourse.bass as bass  # noqa: F401
from concourse import bacc
import concourse.tile as tile
import concourse.mybir as mybir
from concourse.bass_utils import run_bass_kernel_spmd
from concourse.masks import make_identity

B, T, D = 2, 2048, 2048
N, K, H = 16, 4, 128
G = N // K
HALF = H // 2
MIN_TS, MAX_TS = 1.0, 10000.0

P = 128
TCH = 512
NCH = T // TCH          # 4 t-chunks of 512
DB = D // P             # 16 contraction blocks
F32 = mybir.dt.float32
F32R = mybir.dt.float32r
BF16 = mybir.dt.bfloat16
FP16 = mybir.dt.float16
NEG = -1.0e30
EXP = mybir.ActivationFunctionType.Exp
BF = ml_dtypes.bfloat16

_CACHE = {}
LAST = None             # BassKernelResults of the most recent run


def _build():
    if "nc" in _CACHE:
        return _CACHE["nc"]
    nc = bacc.Bacc(None, target_bir_lowering=False, debug=False)
    xq = nc.declare_dram_parameter("xqT", [D, T], FP16, isOutput=False)
    xkv = nc.declare_dram_parameter("xkvT", [D, T], FP16, isOutput=False)
    wq = nc.declare_dram_parameter("wq", [D, G * H], FP16, isOutput=False)
    wk = nc.declare_dram_parameter("wk", [D, H], FP16, isOutput=False)
    wv = nc.declare_dram_parameter("wv", [D, H], FP16, isOutput=False)
    wo = nc.declare_dram_parameter("wo", [G, H, D], BF16, isOutput=False)
    cq = nc.declare_dram_parameter("cosq", [P, T], FP16, isOutput=False)
    sq = nc.declare_dram_parameter("sinq", [P, T], FP16, isOutput=False)
    tri = nc.declare_dram_parameter("tri", [P, P], BF16, isOutput=False)
    o32 = nc.declare_dram_parameter("ones32", [P, 32], BF16, isOutput=False)
    onef = nc.declare_dram_parameter("one_r", [1, P], F32R, isOutput=False)
    pi = nc.declare_dram_parameter("pi", [P, P], FP16, isOutput=False)
    out = nc.declare_dram_parameter("O", [T, D], F32, isOutput=True)

    xq_v = xq[:].rearrange("(do di) t -> di do t", di=P)
    xkv_v = xkv[:].rearrange("(do di) t -> di do t", di=P)
    wq_v = wq[:].rearrange("(do di) nh -> di do nh", di=P)
    wk_v = wk[:].rearrange("(do di) h -> di do h", di=P)
    wv_v = wv[:].rearrange("(do di) h -> di do h", di=P)
    wo_v = wo[:].rearrange("n h d -> h n d")

    with tile.TileContext(nc) as tc:
        with tc.tile_pool(name="glob", bufs=1) as glob, \
             tc.tile_pool(name="xp", bufs=1) as xp, \
             tc.tile_pool(name="dstp", bufs=4) as dstp, \
             tc.tile_pool(name="prodp", bufs=4) as prodp, \
             tc.tile_pool(name="pbp", bufs=6) as pbp, \
             tc.tile_pool(name="otcp", bufs=2) as otcp, \
             tc.tile_pool(name="invp", bufs=2) as invp, \
             tc.tile_pool(name="osbp", bufs=2) as osbp, \
             tc.tile_pool(name="ps_ot", bufs=2, space="PSUM") as ps_ot, \
             tc.tile_pool(name="ps_sc", bufs=2, space="PSUM") as ps_sc, \
             tc.tile_pool(name="ps_den", bufs=2, space="PSUM") as ps_den, \
             tc.tile_pool(name="ps_fx", bufs=2, space="PSUM") as ps_fx:
            qt = glob.tile([P, G, T], FP16)
            kt = glob.tile([P, T], FP16)
            vsb = glob.tile([P, DB, H], BF16)
            wq_sb = glob.tile([P, DB, G * H], FP16)
            wk_sb = glob.tile([P, DB, H], FP16)
            wv_sb = glob.tile([P, DB, H], FP16)
            wo_sb = glob.tile([P, G, D], BF16)
            cosq_sb = glob.tile([P, T], FP16)
            sinq_sb = glob.tile([P, T], FP16)
            tri_sb = glob.tile([P, P], BF16)
            ones32_sb = glob.tile([P, 32], BF16)
            one_r = glob.tile([1, P], F32R)
            pi_sb = glob.tile([P, P], FP16)
            ident = glob.tile([P, P], BF16)
            zer = glob.tile([P, P], FP16)

            make_identity(nc, ident[:])
            nc.gpsimd.memset(zer[:], 0.0)
            # gpsimd queue: tiny tables, then weights in first-use order
            nc.gpsimd.dma_start(tri_sb[:], tri[:])
            nc.gpsimd.dma_start(ones32_sb[:], o32[:])
            nc.gpsimd.dma_start(one_r[:], onef[:])
            nc.gpsimd.dma_start(pi_sb[:], pi[:])
            for db in range(DB):
                nc.gpsimd.dma_start(wq_sb[:, db], wq_v[:, db])
            nc.gpsimd.dma_start(cosq_sb[:], cq[:])
            nc.gpsimd.dma_start(sinq_sb[:], sq[:])
            nc.gpsimd.dma_start(wk_sb[:], wk_v)
            nc.gpsimd.dma_start(wv_sb[:], wv_v)
            nc.gpsimd.dma_start(wo_sb[:], wo_v)

            # warm the PE HAM while initial DMAs land
            warm_ps = ps_fx.tile([P, TCH], F32, tag="fx")
            for _ in range(24):
                nc.tensor.matmul(warm_ps[:, :P], ident[:], ident[:],
                                 start=True, stop=True)

            def rope(ps, out_sl, tsl):
                """out_sl[128,TCH] (fp16) = ps*cos + rot(ps)*sin."""
                dst = dstp.tile([P, TCH], FP16, tag="dst")
                nc.scalar.copy(dst[:], ps[:])
                rot = ps_fx.tile([P, TCH], F32, tag="fx")
                nc.tensor.matmul(rot[:], pi_sb[:], dst[:],
                                 start=True, stop=True)
                prod = prodp.tile([P, TCH], FP16, tag="prod")
                nc.vector.tensor_mul(prod[:], rot[:], sinq_sb[:, tsl])
                nc.vector.tensor_mul(out_sl, dst[:], cosq_sb[:, tsl])
                nc.vector.tensor_add(out_sl, out_sl, prod[:])

            xtiles = {}

            def emit_xdma(c):
                tsl = slice(c * TCH, (c + 1) * TCH)
                xq_sb = xp.tile([P, DB, TCH], FP16, tag="xq", bufs=3,
                                name="xq_sb")
                xkv_sb = xp.tile([P, DB, TCH], FP16, tag="xkv", bufs=2,
                                 name="xkv_sb")
                if c == 0:
                    for q4 in range(4):
                        nc.sync.dma_start(xq_sb[:, 4 * q4:4 * q4 + 4],
                                          xq_v[:, 4 * q4:4 * q4 + 4, tsl])
                    for q4 in range(4):
                        nc.sync.dma_start(xkv_sb[:, 4 * q4:4 * q4 + 4],
                                          xkv_v[:, 4 * q4:4 * q4 + 4, tsl])
                else:
                    nc.sync.dma_start(xq_sb[:], xq_v[:, :, tsl])
                    nc.sync.dma_start(xkv_sb[:], xkv_v[:, :, tsl])
                xtiles[c] = (xq_sb, xkv_sb)

            def emit_projq(c, heads):
                tsl = slice(c * TCH, (c + 1) * TCH)
                xq_sb = xtiles[c][0]
                for n in heads:
                    ps = ps_fx.tile([P, TCH], F32, tag="fx", name="ps")
                    for db in range(DB):
                        nc.tensor.matmul(
                            ps[:], wq_sb[:, db, n * H:(n + 1) * H],
                            xq_sb[:, db, :],
                            start=(db == 0), stop=(db == DB - 1))
                    rope(ps, qt[:, n, tsl], tsl)

            def emit_projkv(c):
                tsl = slice(c * TCH, (c + 1) * TCH)
                xkv_sb = xtiles[c][1]
                ps = ps_fx.tile([P, TCH], F32, tag="fx", name="ps")
                for db in range(DB):
                    nc.tensor.matmul(ps[:], wk_sb[:, db, :], xkv_sb[:, db, :],
                                     start=(db == 0), stop=(db == DB - 1))
                rope(ps, kt[:, tsl], tsl)
                ps2 = ps_fx.tile([P, TCH], F32, tag="fx", name="ps2")
                for db in range(DB):
                    nc.tensor.matmul(ps2[:], wv_sb[:, db, :], xkv_sb[:, db, :],
                                     start=(db == 0), stop=(db == DB - 1))
                vt = dstp.tile([P, TCH], BF16, tag="vt", bufs=2)
                nc.scalar.copy(vt[:], ps2[:])
                for kk in range(4):
                    pst = ps_fx.tile([P, P], BF16, tag="fx", name="pst")
                    nc.tensor.transpose(pst[:], vt[:, kk * P:(kk + 1) * P],
                                        ident[:])
                    nc.scalar.copy(vsb[:, 4 * c + kk, :], pst[:])

            def emit_attn_pair(c, p, otc):
                tsl = slice(c * TCH, (c + 1) * TCH)
                J = 4 * (c + 1)
                heads = (2 * p, 2 * p + 1)
                den_ps = ps_den.tile([P, TCH], F32, tag="den", name="den")
                nc.tensor.matmul(den_ps[:], zer[:], cosq_sb[:, 0:TCH],
                                 start=True, stop=False)
                ots = [ps_ot.tile([P, TCH], F32, tag="ot", name="ot")
                       for _ in heads]
                for j in range(J):
                    d = j - 4 * c
                    lo = max(d, 0) * P
                    pbs = []
                    for i, h in enumerate(heads):
                        sc = ps_sc.tile([P, TCH], F32, tag="sc", name="sc")
                        if d >= 0:
                            nc.tensor.matmul(
                                sc[:, lo:lo + P], ident[:], tri_sb[:],
                                start=True, stop=False)
                            nc.tensor.matmul(
                                sc[:, lo:], kt[:, j * P:(j + 1) * P],
                                qt[:, h, c * TCH + lo:(c + 1) * TCH],
                                start=False, stop=True)
                        else:
                            nc.tensor.matmul(
                                sc[:], kt[:, j * P:(j + 1) * P],
                                qt[:, h, tsl],
                                start=True, stop=True)
                        pb = pbp.tile([P, TCH], BF16, tag="pb", name="pb")
                        nc.scalar.activation(pb[:, lo:], sc[:, lo:], EXP)
                        pbs.append(pb)
                    for i, h in enumerate(heads):
                        nc.tensor.matmul(
                            ots[i][:, lo:], vsb[:, j, :], pbs[i][:, lo:],
                            start=(j == 0), stop=(j == J - 1))
                    for i, h in enumerate(heads):
                        nc.tensor.matmul(
                            den_ps[32 * i:32 * i + 32, lo:],
                            ones32_sb[:], pbs[i][:, lo:],
                            start=False,
                            stop=(j == J - 1 and i == 1),
                            tile_position=(0, 32 * i))
                # normalize: one reciprocal over both heads' 64 rows
                inv = invp.tile([64, TCH], F32, tag="inv", name="inv")
                nc.vector.reciprocal_approx_fast(
                    out=inv[:], in_=den_ps[0:64, :])
                for i, h in enumerate(heads):
                    invr = invp.tile([1, TCH], F32R, tag="invr", name="invr")
                    nc.vector.tensor_copy(invr[:], inv[32 * i:32 * i + 1, :])
                    bc = ps_fx.tile([P, TCH], F32, tag="fx", name="bc")
                    nc.tensor.matmul(bc[:], one_r[:], invr[:],
                                     start=True, stop=True)
                    bcb = prodp.tile([P, TCH], BF16, tag="bcb", name="bcb")
                    nc.scalar.copy(bcb[:], bc[:])
                    nc.vector.tensor_mul(otc[:, h, :], ots[i][:], bcb[:])

            def emit_oproj(c, otc):
                for kk in range(4):
                    row = c * TCH + kk * P
                    osb = osbp.tile([P, D], F32, tag="osb", name="osb")
                    for dc in range(4):
                        ops = ps_fx.tile([P, TCH], F32, tag="fx", name="ops")
                        for n in range(G):
                            nc.tensor.matmul(
                                ops[:],
                                otc[:, n, kk * P:(kk + 1) * P],
                                wo_sb[:, n, dc * TCH:(dc + 1) * TCH],
                                start=(n == 0), stop=(n == G - 1))
                        nc.vector.tensor_copy(
                            osb[:, dc * TCH:(dc + 1) * TCH], ops[:])
                    nc.sync.dma_start(out[row:row + P, :], osb[:])

            # ---- software-pipelined emission ----
            emit_xdma(0)
            emit_projq(0, range(G))
            emit_projkv(0)
            for c in range(NCH):
                if c + 1 < NCH:
                    emit_xdma(c + 1)
                otc = otcp.tile([P, G, TCH], BF16, tag="otc", name="otc")
                emit_attn_pair(c, 0, otc)
                if c + 1 < NCH:
                    emit_projq(c + 1, (0, 1))
                emit_attn_pair(c, 1, otc)
                if c + 1 < NCH:
                    emit_projq(c + 1, (2, 3))
                    emit_projkv(c + 1)
                emit_oproj(c, otc)

    nc.compile()
    _CACHE["nc"] = nc
    return nc


def _rope_tables(pos):
    ts = MIN_TS * (MAX_TS / MIN_TS) ** (2.0 * np.arange(HALF) / H)
    ang = pos.astype(np.float64)[None, :] / ts[:, None]
    c, s = np.cos(ang), np.sin(ang)
    cosF = np.ascontiguousarray(np.concatenate([c, c], 0)).astype(np.float16)
    sinF = np.ascontiguousarray(np.concatenate([-s, s], 0)).astype(np.float16)
    return cosF, sinF


def kernel(Xq, Xkv, q_positions, kv_positions, Wq, Wk, Wv, Wo, _trace=False):
    global LAST
    nc = _build()
    Xq = np.asarray(Xq, dtype=np.float32)
    Xkv = np.asarray(Xkv, dtype=np.float32)
    Wq = np.asarray(Wq, dtype=np.float32)
    Wk = np.asarray(Wk, dtype=np.float32)
    Wv = np.asarray(Wv, dtype=np.float32)
    Wo = np.asarray(Wo, dtype=np.float32)
    qp = np.asarray(q_positions)
    kp = np.asarray(kv_positions)
    assert np.array_equal(qp, kp), (
        "kernel assumes q_positions == kv_positions (RoPE tables shared)")

    idx = np.arange(P)
    tri_np = np.where(idx[:, None] <= idx[None, :], 0.0, NEG).astype(BF)
    pi_np = np.zeros((P, P), np.float32)
    pi_np[(idx + HALF) % P, idx] = 1.0
    pi_np = pi_np.astype(np.float16)
    ones32_np = np.ones((P, 32), BF)
    one_r_np = np.ones((1, P), np.float32)

    xqT = [np.ascontiguousarray(Xq[b].T).astype(np.float16) for b in range(B)]
    xkvT = [np.ascontiguousarray(Xkv[b].T).astype(np.float16)
            for b in range(B)]
    ctabs = [_rope_tables(qp[b]) for b in range(B)]
    wqs = [np.ascontiguousarray(
        Wq[:, kv * G:(kv + 1) * G, :].reshape(D, G * H)).astype(np.float16)
        for kv in range(K)]
    wks = [np.ascontiguousarray(Wk[:, kv, :]).astype(np.float16)
           for kv in range(K)]
    wvs = [np.ascontiguousarray(Wv[:, kv, :]).astype(np.float16)
           for kv in range(K)]
    wos = [np.ascontiguousarray(Wo[kv * G:(kv + 1) * G]).astype(BF)
           for kv in range(K)]

    in_maps = []
    for core in range(8):
        b, kv = divmod(core, 4)
        in_maps.append({
            "xqT": xqT[b],
            "xkvT": xkvT[b],
            "wq": wqs[kv],
            "wk": wks[kv],
            "wv": wvs[kv],
            "wo": wos[kv],
            "cosq": ctabs[b][0], "sinq": ctabs[b][1],
            "tri": tri_np,
            "ones32": ones32_np,
            "one_r": one_r_np,
            "pi": pi_np,
        })

    LAST = run_bass_kernel_spmd(nc, in_maps, list(range(8)), trace=_trace)
    parts = [r["O"] for r in LAST.results]
    O = np.stack([parts[0] + parts[1] + parts[2] + parts[3],
                  parts[4] + parts[5] + parts[6] + parts[7]])
    return np.ascontiguousarray(O.astype(np.float32))
